# revision 22
# baseline (speedup 1.0000x reference)
"""Trainium2 Bass kernel for nn_EqStftSnsePBC (STFT -> per-tap nonlinear PBC -> ISTFT).

Strategy (8 NeuronCores, data parallel over STFT frames):
  host:   frame the signal (hop 216, n_fft 256), split each frame into even/odd
          time samples (radix-2 DIT), pack per-block DRAM buffers that mirror
          the SBUF tile layout exactly (one contiguous DMA per block each way).
  device: per block of NB=116 frames (modes packed along columns, 2NB=232):
            E  = W128 @ xe, F = (diag(w) W128) @ xo      (8 matmuls, K=128)
            X_lo = E + F, X_hi = E - F                   (DVE butterfly == PSUM copy)
            I  = sum_modes |X|^2                         (ACT square + DVE folds)
            phi = G @ I  (circulant corr, -P folded)     (8 matmuls)
            U  = j*P*phi .* X                            (DVE elementwise)
            v  = IDFT256 @ U  (dense, 2 row-chunks)      (16 matmuls)
          all bf16 with f32 PSUM accumulation.
  host:   yf = frames + v (exact passthrough of linear term), overlap-add,
          divide by coverage, trim, restack.
"""

import os
import sys

for _p in ("/opt/trn_rl_repo",):
    if os.path.isdir(_p) and _p not in sys.path:
        sys.path.append(_p)

import numpy as np
try:
    import ml_dtypes
    _BF16 = np.dtype(ml_dtypes.bfloat16)
except Exception:
    _BF16 = None

# ---- problem geometry (hardcoded) ----
MTAPS = 41
PAD = MTAPS // 2  # 20
NFFT = 256
HOP = 216
B = 2
NM = 2
L = 999688
STEPS = 4628            # (L - NFFT) // HOP + 1
NCORES = 8
NB = 116                # frames per block
NQ = 5                  # blocks per (core, b)
NH = NQ * NB            # 580 frames per core per b
FTOT = NCORES * NH      # 4640 >= STEPS (12 trailing fake frames, ignored on host)
NBLK = B * NQ           # 10 blocks per core
W2 = 2 * NB             # 232  (modes packed)
W4 = 4 * NB             # 464

_PROG = None
LAST_EXEC_NS = None
LAST_RESULTS = None


def _build_const_matrices(h_real, h_imag, task_info):
    """All lhsT constant matrices, bf16-packed for the device.

    wall [128, 18*128]: radix-2 FFT mats (ME, MF) and dense IDFT blocks,
      each as (Mr, Mi, -Mi) triples of lhsT = M.T.
    gall [128, 16*128]: correlation lhsT blocks per (b, ri, half, kc),
      scaled by -P[b].
    """
    n = np.arange(128)
    W128 = np.exp(-2j * np.pi * np.outer(n, n) / 128.0)        # [k, n]
    w = np.exp(-2j * np.pi * np.arange(128) / 256.0)           # twiddles
    ME = W128                                                   # E = ME @ xe
    MF = w[:, None] * W128                                      # F = MF @ xo
    t = np.arange(NFFT)
    IDFT = np.exp(2j * np.pi * np.outer(t, t) / NFFT) / NFFT    # [t, k]

    mats = [ME, MF]
    for tc in range(2):
        for kc in range(2):
            mats.append(IDFT[tc * 128:(tc + 1) * 128, kc * 128:(kc + 1) * 128])
    # -> 6 complex matrices -> 18 real lhsT blocks (Mr, Mi, -Mi each)
    wall = np.empty((18, 128, 128), np.float32)
    for i, M in enumerate(mats):
        lr, li = M.T.real, M.T.imag    # lhsT[n, k] = M[k, n]
        wall[3 * i + 0] = lr
        wall[3 * i + 1] = li
        wall[3 * i + 2] = -li

    # G[p', m] circulant correlation matrix (same as reference semantics)
    def toep(h):
        G = np.zeros((NFFT, NFFT), np.float64)
        for p in range(NFFT + 2 * PAD):
            pp = (p - PAD) % NFFT
            lo, hi = max(0, p - (MTAPS - 1)), min(NFFT - 1, p)
            if lo <= hi:
                ms = np.arange(lo, hi + 1)
                G[pp, ms] += h[p - ms]
        return G

    Gr = toep(np.asarray(h_real, np.float64))
    Gi = toep(np.asarray(h_imag, np.float64))
    P = 10.0 ** (np.asarray(task_info, np.float64)[:, 0] / 10.0) / NM
    # toep's G is [source_freq, output_freq]: phi = G.T @ I, so the lhsT
    # block for output-half `half`, input-chunk `kc` is G[kc rows, half cols].
    gall = np.empty((B, 2, 2, 2, 128, 128), np.float32)  # [b, ri, half, kc, n, k]
    for b in range(B):
        for ri, G in enumerate((Gr, Gi)):
            for half in range(2):
                for kc in range(2):
                    blk = G[kc * 128:(kc + 1) * 128, half * 128:(half + 1) * 128]
                    gall[b, ri, half, kc] = (-P[b] * blk)
    wall_p = np.ascontiguousarray(
        wall.transpose(1, 0, 2).reshape(128, 18 * 128)).astype(_BF16)
    gall_p = np.ascontiguousarray(
        gall.reshape(16, 128, 128).transpose(1, 0, 2).reshape(128, 16 * 128)
    ).astype(_BF16)
    return wall, gall, wall_p, gall_p


def _frames_view(x_real, x_imag):
    """-> F [B, NM, 2, FTOT, NFFT] float32 frames (zero-padded past L)."""
    need = HOP * (FTOT - 1) + NFFT
    F = np.empty((B, NM, 2, FTOT, NFFT), np.float32)
    for ri, x in enumerate((x_real, x_imag)):
        xt = np.ascontiguousarray(np.asarray(x, np.float32).transpose(0, 2, 1))
        xp = np.zeros((B, NM, need), np.float32)
        xp[:, :, :L] = xt
        sw = np.lib.stride_tricks.as_strided(
            xp, shape=(B, NM, FTOT, NFFT),
            strides=(xp.strides[0], xp.strides[1], HOP * 4, 4))
        F[:, :, ri] = sw
    return F


def _pack_inputs(F):
    """F [B,NM,2,FTOT,256] -> per-core xin [NCORES][NBLK,128,4,NM,NB] bf16.

    s index = eo*2 + ri: 0=(even,re) 1=(even,im) 2=(odd,re) 3=(odd,im).
    """
    xe = F[..., 0::2]   # [B, NM, 2, FTOT, 128]
    xo = F[..., 1::2]
    arr = np.stack([xe, xo], axis=0)  # [eo, B, NM, ri, FTOT, 128]
    r1 = arr.reshape(2, B, NM, 2, NCORES, NQ, NB, 128)
    # -> [k, b, q, n, eo, ri, m, j]
    out = r1.transpose(4, 1, 5, 7, 0, 3, 2, 6)
    out = np.ascontiguousarray(out).astype(_BF16)
    return out.reshape(NCORES, NBLK, 128, 4, NM, NB)


def _unpack_outputs(vouts):
    """vouts [NCORES][NBLK,128,2,2,NM,NB] -> v frames [B,NM,2,FTOT,256] f32."""
    va = np.stack([v.astype(np.float32) for v in vouts], axis=0)
    # dims [k, b, q, n, tc, ri, m, j] -> [b, m, ri, k, q, j, tc, n]
    va = va.reshape(NCORES, B, NQ, 128, 2, 2, NM, NB)
    vfr = va.transpose(1, 6, 5, 0, 2, 7, 4, 3).reshape(B, NM, 2, FTOT, NFFT)
    return vfr


def _overlap_add(yf):
    """yf [B, NM, 2, NFFT, FTOT] -> y [B, NM, 2, L] (OLA / coverage)."""
    y = np.zeros((B, NM, 2, STEPS, HOP), np.float32)
    body = yf[:, :, :, :HOP, :STEPS].transpose(0, 1, 2, 4, 3)
    y[:] = body
    tail = yf[:, :, :, HOP:, :STEPS - 1].transpose(0, 1, 2, 4, 3)
    y[:, :, :, 1:, :NFFT - HOP] += tail
    y = y.reshape(B, NM, 2, STEPS * HOP)
    yfull = np.empty((B, NM, 2, L), np.float32)
    yfull[:, :, :, :STEPS * HOP] = y
    yfull[:, :, :, STEPS * HOP:] = yf[:, :, :, HOP:HOP + (L - STEPS * HOP), STEPS - 1]
    t = np.arange(L)
    wsum = np.ones(L, np.float32)
    wsum[(t >= HOP) & (t < STEPS * HOP) & (t % HOP < NFFT - HOP)] = 2.0
    yfull /= wsum
    return yfull


def _build_program():
    import concourse.bass as bass
    import concourse.tile as tile
    from concourse import bacc, mybir
    from contextlib import ExitStack

    f32 = mybir.dt.float32
    bf16 = mybir.dt.bfloat16
    MULT = mybir.AluOpType.mult
    ADD = mybir.AluOpType.add
    SUB = mybir.AluOpType.subtract
    SQUARE = mybir.ActivationFunctionType.Square

    nc = bacc.Bacc(None, target_bir_lowering=False, debug=False)
    xin_d = nc.dram_tensor("xin", [NBLK, 128, 8, NB], bf16,
                           kind="ExternalInput").ap()
    wall_d = nc.dram_tensor("wall", [128, 18 * 128], bf16,
                            kind="ExternalInput").ap()
    gall_d = nc.dram_tensor("gall", [128, 16 * 128], bf16,
                            kind="ExternalInput").ap()
    vout_d = nc.dram_tensor("vout", [NBLK, 128, 2, 2 * W2], bf16,
                            kind="ExternalOutput").ap()

    # wall block index: (mat, part) mat in [ME, MF, I00, I01, I10, I11],
    # part in [r, i, negi]
    def wslice(wall_sb, mat, part):
        off = (mat * 3 + part) * 128
        return wall_sb[:, off:off + 128]

    def gslice(gall_sb, b, ri, half, kc):
        off = (((b * 2 + ri) * 2 + half) * 2 + kc) * 128
        return gall_sb[:, off:off + 128]

    with tile.TileContext(nc) as tc:
        with ExitStack() as ctx:
            consts = ctx.enter_context(tc.tile_pool(name="consts", bufs=1))
            xin_p = ctx.enter_context(tc.tile_pool(name="xin", bufs=4))
            xsb_p = ctx.enter_context(tc.tile_pool(name="xsb", bufs=3))
            isb_p = ctx.enter_context(tc.tile_pool(name="isb", bufs=3))
            dup_p = ctx.enter_context(tc.tile_pool(name="dup", bufs=2))
            usb_p = ctx.enter_context(tc.tile_pool(name="usb", bufs=2))
            osb_p = ctx.enter_context(tc.tile_pool(name="osb", bufs=2))
            ps_fft = ctx.enter_context(tc.tile_pool(name="psf", bufs=2, space="PSUM"))
            ps_cor = ctx.enter_context(tc.tile_pool(name="psc", bufs=1, space="PSUM"))
            ps_ift = ctx.enter_context(tc.tile_pool(name="psv", bufs=3, space="PSUM"))

            # const loads: FFT matrices first on the (early-idle) gpsimd
            # queue; the rest follow there. Input DMAs own the sync queue.
            wall = consts.tile([128, 18 * 128], bf16, tag="wall")
            nc.gpsimd.dma_start(wall[:, :6 * 128], wall_d[:, :6 * 128])
            gall = consts.tile([128, 16 * 128], bf16, tag="gall")
            nc.scalar.dma_start(gall[:], gall_d[:])
            nc.scalar.dma_start(wall[:, 6 * 128:], wall_d[:, 6 * 128:])

            state = {}

            def eDMA(t):
                xin = xin_p.tile([128, 8, NB], bf16, tag="xin", name=f"xin{t}")
                if t < 2:
                    # first blocks: split across two queues for parallel transfer
                    nc.sync.dma_start(xin[:, 0:4], xin_d[t, :, 0:4])
                    nc.scalar.dma_start(xin[:, 4:8], xin_d[t, :, 4:8])
                else:
                    nc.sync.dma_start(xin[:], xin_d[t])
                state[t] = {"xin": xin}

            def eA(t):
                """FFT matmuls + butterfly combine -> X in SBUF (flat [128,464])."""
                xin = state[t]["xin"]
                E = ps_fft.tile([128, 4, NB], f32, tag="E", name=f"E{t}")
                Fp = ps_fft.tile([128, 4, NB], f32, tag="F", name=f"F{t}")
                for (ps, mat, ur, ui) in ((E, 0, xin[:, 0:2], xin[:, 2:4]),
                                          (Fp, 1, xin[:, 4:6], xin[:, 6:8])):
                    mr = wslice(wall, mat, 0)
                    mi = wslice(wall, mat, 1)
                    mni = wslice(wall, mat, 2)
                    nc.tensor.matmul(ps[:, 0:2], mr, ur, start=True, stop=False)
                    nc.tensor.matmul(ps[:, 0:2], mni, ui, start=False, stop=True)
                    nc.tensor.matmul(ps[:, 2:4], mr, ui, start=True, stop=False)
                    nc.tensor.matmul(ps[:, 2:4], mi, ur, start=False, stop=True)
                # butterfly: X_lo = E + F, X_hi = E - F  (to SBUF bf16).
                # TT can read only one PSUM operand, so stage F through SBUF.
                Fs = xsb_p.tile([128, 4, NB], bf16, tag="Fs", name=f"Fs{t}")
                nc.scalar.copy(Fs[:], Fp[:])
                Xlo = xsb_p.tile([128, 4, NB], bf16, tag="Xlo", name=f"Xlo{t}")
                Xhi = xsb_p.tile([128, 4, NB], bf16, tag="Xhi", name=f"Xhi{t}")
                nc.vector.tensor_tensor(Xlo[:], E[:], Fs[:], ADD)
                nc.vector.tensor_tensor(Xhi[:], E[:], Fs[:], SUB)
                state[t].update({"Xlo": Xlo, "Xhi": Xhi})

            def eB(t):
                """intensity: sq (ACT), ri-fold (DVE) + mode-fold (GPS)."""
                st = state[t]
                I = isb_p.tile([128, 2, NB], bf16, tag="I", name=f"I{t}")
                for h, X in enumerate((st["Xlo"], st["Xhi"])):
                    sq = isb_p.tile([128, 4, NB], bf16, tag=f"sq{h}",
                                    name=f"sq{t}_{h}")
                    nc.scalar.activation(sq[:], X[:], SQUARE)
                    s = isb_p.tile([128, 2, NB], bf16, tag=f"s{h}", name=f"s{t}_{h}")
                    nc.gpsimd.tensor_tensor(s[:], sq[:, 0:2], sq[:, 2:4], ADD)
                    nc.gpsimd.tensor_tensor(I[:, h], s[:, 0], s[:, 1], ADD)
                st["I"] = I

            def eC(t):
                """corr matmuls -> phi psum bank; single bf16 copy (no dup)."""
                b = t // NQ
                st = state[t]
                I = st["I"]
                # phi bank [128, 4, NB]: q = ri*2 + half
                ph = ps_cor.tile([128, 4, 1, NB], f32, tag="ph", name=f"ph{t}")
                for ri in range(2):
                    for half in range(2):
                        q = ri * 2 + half
                        nc.tensor.matmul(ph[:, q], gslice(gall, b, ri, half, 0),
                                         I[:, 0], start=True, stop=False)
                        nc.tensor.matmul(ph[:, q], gslice(gall, b, ri, half, 1),
                                         I[:, 1], start=False, stop=True)
                phs = dup_p.tile([128, 4, 1, NB], bf16, tag="phs", name=f"phs{t}")
                nc.scalar.copy(phs[:], ph[:])
                st["phs"] = phs

            def eD(t):
                """U = j*P*phi .* X via broadcast reads of phi.

                X_h [128,4,NB] = (ri*m, j); T0 = nb*X, T1 = na*X (phi bcast
                over ri,m); Ur = T0[r]+T1[i], Ui = T0[i]-T1[r].
                """
                st = state[t]
                phs = st["phs"]
                U = {}
                for h, X in enumerate((st["Xlo"], st["Xhi"])):
                    naB = phs[:, 0 + h].broadcast_to([128, 4, NB])
                    nbB = phs[:, 2 + h].broadcast_to([128, 4, NB])
                    T0 = usb_p.tile([128, 4, NB], bf16, tag=f"T0{h}",
                                    name=f"T0{t}_{h}")
                    T1 = usb_p.tile([128, 4, NB], bf16, tag=f"T1{h}",
                                    name=f"T1{t}_{h}")
                    nc.vector.tensor_tensor(T0[:], X[:], nbB, MULT)
                    nc.vector.tensor_tensor(T1[:], X[:], naB, MULT)
                    Uh = usb_p.tile([128, 4, NB], bf16, tag=f"U{h}",
                                    name=f"U{t}_{h}")
                    nc.vector.tensor_tensor(Uh[:, 0:2], T0[:, 0:2], T1[:, 2:4], ADD)
                    nc.vector.tensor_tensor(Uh[:, 2:4], T0[:, 2:4], T1[:, 0:2], SUB)
                    U[h] = Uh
                st["U"] = U

            def eE(t):
                """dense IFFT: v_tc = sum_kc IDFT[tc,kc] @ U_kc, out copy + DMA."""
                st = state[t]
                U = st["U"]
                ob = osb_p.tile([128, 2, 2 * W2], bf16, tag="ob", name=f"ob{t}")
                for tcn in range(2):
                    vp = ps_ift.tile([128, 2 * W2], f32, tag="vp",
                                     name=f"vp{t}_{tcn}")
                    mats = [(2 + tcn * 2 + kc) for kc in range(2)]
                    seq_r = []
                    seq_i = []
                    for kc in range(2):
                        mat = mats[kc]
                        Ur, Ui = U[kc][:, 0:2], U[kc][:, 2:4]
                        seq_r += [(wslice(wall, mat, 0), Ur),
                                  (wslice(wall, mat, 2), Ui)]
                        seq_i += [(wslice(wall, mat, 0), Ui),
                                  (wslice(wall, mat, 1), Ur)]
                    for ri, seq in enumerate((seq_r, seq_i)):
                        for i, (lhsT, rhs) in enumerate(seq):
                            nc.tensor.matmul(vp[:, ri * W2:(ri + 1) * W2],
                                             lhsT, rhs,
                                             start=(i == 0), stop=(i == 3))
                    nc.scalar.copy(ob[:, tcn], vp[:])
                if t == NBLK - 1:
                    # last block: split across queues so the tail drains fast
                    nc.gpsimd.dma_start(vout_d[t, :, 0], ob[:, 0])
                    nc.sync.dma_start(vout_d[t, :, 1], ob[:, 1])
                else:
                    nc.gpsimd.dma_start(vout_d[t], ob[:])
                del state[t]

            # software pipeline: keep tensor fed two blocks ahead
            eDMA(0); eDMA(1); eDMA(2)
            eA(0); eB(0); eA(1); eB(1)
            for t in range(NBLK):
                eC(t)
                eD(t)
                if t + 3 < NBLK:
                    eDMA(t + 3)
                if t + 2 < NBLK:
                    eA(t + 2)
                    eB(t + 2)
                eE(t)

    nc.compile()
    return nc


def _run_device(xin_cores, wall_p, gall_p, trace=False):
    global _PROG, LAST_EXEC_NS, LAST_RESULTS
    from concourse.bass_utils import run_bass_kernel_spmd

    if _PROG is None:
        _PROG = _build_program()
    nc = _PROG
    in_maps = []
    for k in range(NCORES):
        in_maps.append({
            "xin": np.ascontiguousarray(xin_cores[k]).reshape(NBLK, 128, 8, NB),
            "wall": wall_p,
            "gall": gall_p,
        })
    kwargs = {}
    if trace:
        kwargs["trace"] = True
    res = run_bass_kernel_spmd(nc, in_maps, list(range(NCORES)), **kwargs)
    LAST_EXEC_NS = res.exec_time_ns
    LAST_RESULTS = res
    return [res.results[k]["vout"] for k in range(NCORES)]


def _emulate_device(xin_cores, wall, gall):
    """Numpy mirror of the device program (f32)."""
    outs = []
    for k in range(NCORES):
        xin = xin_cores[k].astype(np.float32)  # [NBLK, 128, 4, NM, NB]
        vout = np.empty((NBLK, 128, 2, 2, NM, NB), np.float32)
        for t in range(NBLK):
            b = t // NQ
            xer, xei = xin[t, :, 0].reshape(128, W2), xin[t, :, 1].reshape(128, W2)
            xor_, xoi = xin[t, :, 2].reshape(128, W2), xin[t, :, 3].reshape(128, W2)
            Er = wall[0].T @ xer + wall[2].T @ xei
            Ei = wall[0].T @ xei + wall[1].T @ xer
            Fr = wall[3].T @ xor_ + wall[5].T @ xoi
            Fi = wall[3].T @ xoi + wall[4].T @ xor_
            X = {0: (Er + Fr, Ei + Fi), 1: (Er - Fr, Ei - Fi)}
            I = {}
            for h in range(2):
                Xr, Xi = X[h]
                s = (Xr * Xr + Xi * Xi).reshape(128, NM, NB)
                I[h] = s[:, 0] + s[:, 1]
            gq = gall.reshape(B, 2, 2, 2, 128, 128)
            U = {}
            for h in range(2):
                na = gq[b, 0, h, 0].T @ I[0] + gq[b, 0, h, 1].T @ I[1]
                nb_ = gq[b, 1, h, 0].T @ I[0] + gq[b, 1, h, 1].T @ I[1]
                na2 = np.repeat(na[:, None, :], NM, 1).reshape(128, W2)
                nb2 = np.repeat(nb_[:, None, :], NM, 1).reshape(128, W2)
                Xr, Xi = X[h]
                U[h] = (nb2 * Xr + na2 * Xi, nb2 * Xi - na2 * Xr)
            for tcn in range(2):
                acc_r = np.zeros((128, W2), np.float32)
                acc_i = np.zeros((128, W2), np.float32)
                for kc in range(2):
                    mat = 2 + tcn * 2 + kc
                    Ur, Ui = U[kc]
                    acc_r += wall[3 * mat].T @ Ur + wall[3 * mat + 2].T @ Ui
                    acc_i += wall[3 * mat].T @ Ui + wall[3 * mat + 1].T @ Ur
                vout[t, :, tcn, 0] = acc_r.reshape(128, NM, NB)
                vout[t, :, tcn, 1] = acc_i.reshape(128, NM, NB)
        outs.append(vout)
    return outs


def kernel(x_real, x_imag, task_info, h_real, h_imag, _emulate=False, _trace=False):
    x_real = np.asarray(x_real, np.float32)
    x_imag = np.asarray(x_imag, np.float32)
    wall, gall, wall_p, gall_p = _build_const_matrices(h_real, h_imag, task_info)
    F = _frames_view(x_real, x_imag)
    xin_cores = _pack_inputs(F)
    if _emulate:
        vouts = _emulate_device(xin_cores, wall, gall)
    else:
        vouts = _run_device(xin_cores, wall_p, gall_p, trace=_trace)
    vfr = _unpack_outputs(vouts)
    yf = (F + vfr).transpose(0, 1, 2, 4, 3)   # [B, NM, 2, NFFT, FTOT]
    y = _overlap_add(yf)
    y = y[:, :, :, PAD:L - PAD]
    return np.ascontiguousarray(y.transpose(0, 3, 1, 2))


# revision 23
# speedup vs baseline: 1.2328x; 1.2328x over previous
"""Trainium2 Bass kernel for nn_EqStftSnsePBC (STFT -> per-tap nonlinear PBC -> ISTFT).

Strategy (8 NeuronCores, data parallel over STFT frames):
  host:   frame the signal (hop 216, n_fft 256), split each frame into even/odd
          time samples (radix-2 DIT), pack per-block DRAM buffers that mirror
          the SBUF tile layout exactly (one contiguous DMA per block each way).
  device: per block of NB=116 frames (modes packed along columns, 2NB=232):
            E  = W128 @ xe, F = (diag(w) W128) @ xo      (8 matmuls, K=128)
            X_lo = E + F, X_hi = E - F                   (DVE butterfly == PSUM copy)
            I  = sum_modes |X|^2                         (ACT square + DVE folds)
            phi = G @ I  (circulant corr, -P folded)     (8 matmuls)
            U  = j*P*phi .* X                            (DVE elementwise)
            v  = IDFT256 @ U  (dense, 2 row-chunks)      (16 matmuls)
          all bf16 with f32 PSUM accumulation.
  host:   yf = frames + v (exact passthrough of linear term), overlap-add,
          divide by coverage, trim, restack.
"""

import os
import sys

for _p in ("/opt/trn_rl_repo",):
    if os.path.isdir(_p) and _p not in sys.path:
        sys.path.append(_p)

import numpy as np
try:
    import ml_dtypes
    _BF16 = np.dtype(ml_dtypes.bfloat16)
except Exception:
    _BF16 = None

# ---- problem geometry (hardcoded) ----
MTAPS = 41
PAD = MTAPS // 2  # 20
NFFT = 256
HOP = 216
B = 2
NM = 2
L = 999688
STEPS = 4628            # (L - NFFT) // HOP + 1
NCORES = 8
NB = 116                # frames per block
NQ = 5                  # blocks per (core, b)
NH = NQ * NB            # 580 frames per core per b
FTOT = NCORES * NH      # 4640 >= STEPS (12 trailing fake frames, ignored on host)
NBLK = B * NQ           # 10 blocks per core
W2 = 2 * NB             # 232  (modes packed)
W4 = 4 * NB             # 464

_PROG = None
LAST_EXEC_NS = None
LAST_RESULTS = None


def _build_const_matrices(h_real, h_imag, task_info):
    """All lhsT constant matrices, bf16-packed for the device.

    wall [128, 18*128]: radix-2 FFT mats (ME, MF) and dense IDFT blocks,
      each as (Mr, Mi, -Mi) triples of lhsT = M.T.
    gall [128, 16*128]: correlation lhsT blocks per (b, ri, half, kc),
      scaled by -P[b].
    """
    n = np.arange(128)
    W128 = np.exp(-2j * np.pi * np.outer(n, n) / 128.0)        # [k, n]
    w = np.exp(-2j * np.pi * np.arange(128) / 256.0)           # twiddles
    ME = W128                                                   # E = ME @ xe
    MF = w[:, None] * W128                                      # F = MF @ xo
    t = np.arange(NFFT)
    IDFT = np.exp(2j * np.pi * np.outer(t, t) / NFFT) / NFFT    # [t, k]

    mats = [ME, MF]
    for tc in range(2):
        for kc in range(2):
            mats.append(IDFT[tc * 128:(tc + 1) * 128, kc * 128:(kc + 1) * 128])
    # -> 6 complex matrices -> 18 real lhsT blocks (Mr, Mi, -Mi each)
    wall = np.empty((18, 128, 128), np.float32)
    for i, M in enumerate(mats):
        lr, li = M.T.real, M.T.imag    # lhsT[n, k] = M[k, n]
        wall[3 * i + 0] = lr
        wall[3 * i + 1] = li
        wall[3 * i + 2] = -li

    # G[p', m] circulant correlation matrix (same as reference semantics)
    def toep(h):
        G = np.zeros((NFFT, NFFT), np.float64)
        for p in range(NFFT + 2 * PAD):
            pp = (p - PAD) % NFFT
            lo, hi = max(0, p - (MTAPS - 1)), min(NFFT - 1, p)
            if lo <= hi:
                ms = np.arange(lo, hi + 1)
                G[pp, ms] += h[p - ms]
        return G

    Gr = toep(np.asarray(h_real, np.float64))
    Gi = toep(np.asarray(h_imag, np.float64))
    P = 10.0 ** (np.asarray(task_info, np.float64)[:, 0] / 10.0) / NM
    # toep's G is [source_freq, output_freq]: phi = G.T @ I, so the lhsT
    # block for output-half `half`, input-chunk `kc` is G[kc rows, half cols].
    gall = np.empty((B, 2, 2, 2, 128, 128), np.float32)  # [b, ri, half, kc, n, k]
    for b in range(B):
        for ri, G in enumerate((Gr, Gi)):
            for half in range(2):
                for kc in range(2):
                    blk = G[kc * 128:(kc + 1) * 128, half * 128:(half + 1) * 128]
                    gall[b, ri, half, kc] = (-P[b] * blk)
    wall_p = np.ascontiguousarray(
        wall.transpose(1, 0, 2).reshape(128, 18 * 128)).astype(_BF16)
    gall_p = np.ascontiguousarray(
        gall.reshape(16, 128, 128).transpose(1, 0, 2).reshape(128, 16 * 128)
    ).astype(_BF16)
    return wall, gall, wall_p, gall_p


def _frames_view(x_real, x_imag):
    """-> F [B, NM, 2, FTOT, NFFT] float32 frames (zero-padded past L)."""
    need = HOP * (FTOT - 1) + NFFT
    F = np.empty((B, NM, 2, FTOT, NFFT), np.float32)
    for ri, x in enumerate((x_real, x_imag)):
        xt = np.ascontiguousarray(np.asarray(x, np.float32).transpose(0, 2, 1))
        xp = np.zeros((B, NM, need), np.float32)
        xp[:, :, :L] = xt
        sw = np.lib.stride_tricks.as_strided(
            xp, shape=(B, NM, FTOT, NFFT),
            strides=(xp.strides[0], xp.strides[1], HOP * 4, 4))
        F[:, :, ri] = sw
    return F


def _pack_inputs(F):
    """F [B,NM,2,FTOT,256] -> per-core xin [NCORES][NBLK,128,4,NM,NB] bf16.

    s index = eo*2 + ri: 0=(even,re) 1=(even,im) 2=(odd,re) 3=(odd,im).
    """
    xe = F[..., 0::2]   # [B, NM, 2, FTOT, 128]
    xo = F[..., 1::2]
    arr = np.stack([xe, xo], axis=0)  # [eo, B, NM, ri, FTOT, 128]
    r1 = arr.reshape(2, B, NM, 2, NCORES, NQ, NB, 128)
    # -> [k, b, q, n, eo, ri, m, j]
    out = r1.transpose(4, 1, 5, 7, 0, 3, 2, 6)
    out = np.ascontiguousarray(out).astype(_BF16)
    return out.reshape(NCORES, NBLK, 128, 4, NM, NB)


def _unpack_outputs(vouts):
    """vouts [NCORES][NBLK,128,2,2,NM,NB] -> v frames [B,NM,2,FTOT,256] f32."""
    va = np.stack([v.astype(np.float32) for v in vouts], axis=0)
    # dims [k, b, q, n, tc, ri, m, j] -> [b, m, ri, k, q, j, tc, n]
    va = va.reshape(NCORES, B, NQ, 128, 2, 2, NM, NB)
    vfr = va.transpose(1, 6, 5, 0, 2, 7, 4, 3).reshape(B, NM, 2, FTOT, NFFT)
    return vfr


def _overlap_add(yf):
    """yf [B, NM, 2, NFFT, FTOT] -> y [B, NM, 2, L] (OLA / coverage)."""
    y = np.zeros((B, NM, 2, STEPS, HOP), np.float32)
    body = yf[:, :, :, :HOP, :STEPS].transpose(0, 1, 2, 4, 3)
    y[:] = body
    tail = yf[:, :, :, HOP:, :STEPS - 1].transpose(0, 1, 2, 4, 3)
    y[:, :, :, 1:, :NFFT - HOP] += tail
    y = y.reshape(B, NM, 2, STEPS * HOP)
    yfull = np.empty((B, NM, 2, L), np.float32)
    yfull[:, :, :, :STEPS * HOP] = y
    yfull[:, :, :, STEPS * HOP:] = yf[:, :, :, HOP:HOP + (L - STEPS * HOP), STEPS - 1]
    t = np.arange(L)
    wsum = np.ones(L, np.float32)
    wsum[(t >= HOP) & (t < STEPS * HOP) & (t % HOP < NFFT - HOP)] = 2.0
    yfull /= wsum
    return yfull


def _build_program():
    import concourse.bass as bass
    import concourse.tile as tile
    from concourse import bacc, mybir
    from contextlib import ExitStack

    f32 = mybir.dt.float32
    bf16 = mybir.dt.bfloat16
    MULT = mybir.AluOpType.mult
    ADD = mybir.AluOpType.add
    SUB = mybir.AluOpType.subtract
    SQUARE = mybir.ActivationFunctionType.Square

    nc = bacc.Bacc(None, target_bir_lowering=False, debug=False)
    xin_d = nc.dram_tensor("xin", [NBLK, 128, 8, NB], bf16,
                           kind="ExternalInput").ap()
    wall_d = nc.dram_tensor("wall", [128, 18 * 128], bf16,
                            kind="ExternalInput").ap()
    gall_d = nc.dram_tensor("gall", [128, 16 * 128], bf16,
                            kind="ExternalInput").ap()
    vout_d = nc.dram_tensor("vout", [NBLK, 128, 2, 2 * W2], bf16,
                            kind="ExternalOutput").ap()

    # wall block index: (mat, part) mat in [ME, MF, I00, I01, I10, I11],
    # part in [r, i, negi]
    def wslice(wall_sb, mat, part):
        off = (mat * 3 + part) * 128
        return wall_sb[:, off:off + 128]

    def gslice(gall_sb, b, ri, half, kc):
        off = (((b * 2 + ri) * 2 + half) * 2 + kc) * 128
        return gall_sb[:, off:off + 128]

    with tile.TileContext(nc) as tc:
        with ExitStack() as ctx:
            consts = ctx.enter_context(tc.tile_pool(name="consts", bufs=1))
            xin_p = ctx.enter_context(tc.tile_pool(name="xin", bufs=4))
            xsb_p = ctx.enter_context(tc.tile_pool(name="xsb", bufs=3))
            isb_p = ctx.enter_context(tc.tile_pool(name="isb", bufs=3))
            dup_p = ctx.enter_context(tc.tile_pool(name="dup", bufs=2))
            usb_p = ctx.enter_context(tc.tile_pool(name="usb", bufs=2))
            osb_p = ctx.enter_context(tc.tile_pool(name="osb", bufs=2))
            ps_fft = ctx.enter_context(tc.tile_pool(name="psf", bufs=2, space="PSUM"))
            ps_cor = ctx.enter_context(tc.tile_pool(name="psc", bufs=2, space="PSUM"))
            ps_ift = ctx.enter_context(tc.tile_pool(name="psv", bufs=1, space="PSUM"))

            # const loads: FFT matrices first on the (early-idle) gpsimd
            # queue; the rest follow there. Input DMAs own the sync queue.
            wall = consts.tile([128, 18 * 128], bf16, tag="wall")
            nc.gpsimd.dma_start(wall[:, :6 * 128], wall_d[:, :6 * 128])
            gall = consts.tile([128, 16 * 128], bf16, tag="gall")
            nc.scalar.dma_start(gall[:], gall_d[:])
            nc.scalar.dma_start(wall[:, 6 * 128:], wall_d[:, 6 * 128:])

            state = {}

            def eDMA(t):
                xin = xin_p.tile([128, 8, NB], bf16, tag="xin", name=f"xin{t}")
                if t < 2:
                    # first blocks: split across two queues for parallel transfer
                    nc.sync.dma_start(xin[:, 0:4], xin_d[t, :, 0:4])
                    nc.scalar.dma_start(xin[:, 4:8], xin_d[t, :, 4:8])
                else:
                    nc.sync.dma_start(xin[:], xin_d[t])
                state[t] = {"xin": xin}

            def eA(t):
                """FFT matmuls + butterfly combine -> X in SBUF (flat [128,464])."""
                xin = state[t]["xin"]
                E = ps_fft.tile([128, 4, NB], f32, tag="E", name=f"E{t}")
                Fp = ps_fft.tile([128, 4, NB], f32, tag="F", name=f"F{t}")
                for (ps, mat, ur, ui) in ((E, 0, xin[:, 0:2], xin[:, 2:4]),
                                          (Fp, 1, xin[:, 4:6], xin[:, 6:8])):
                    mr = wslice(wall, mat, 0)
                    mi = wslice(wall, mat, 1)
                    mni = wslice(wall, mat, 2)
                    nc.tensor.matmul(ps[:, 0:2], mr, ur, start=True, stop=False)
                    nc.tensor.matmul(ps[:, 0:2], mni, ui, start=False, stop=True)
                    nc.tensor.matmul(ps[:, 2:4], mr, ui, start=True, stop=False)
                    nc.tensor.matmul(ps[:, 2:4], mi, ur, start=False, stop=True)
                # butterfly: X_lo = E + F, X_hi = E - F  (to SBUF bf16).
                # TT can read only one PSUM operand, so stage F through SBUF.
                Fs = xsb_p.tile([128, 4, NB], bf16, tag="Fs", name=f"Fs{t}")
                nc.scalar.copy(Fs[:], Fp[:])
                Xlo = xsb_p.tile([128, 4, NB], bf16, tag="Xlo", name=f"Xlo{t}")
                Xhi = xsb_p.tile([128, 4, NB], bf16, tag="Xhi", name=f"Xhi{t}")
                nc.vector.tensor_tensor(Xlo[:], E[:], Fs[:], ADD)
                nc.vector.tensor_tensor(Xhi[:], E[:], Fs[:], SUB)
                state[t].update({"Xlo": Xlo, "Xhi": Xhi})

            def eB(t):
                """intensity: sq (ACT), ri-fold (DVE) + mode-fold (GPS)."""
                st = state[t]
                I = isb_p.tile([128, 2, NB], bf16, tag="I", name=f"I{t}")
                for h, X in enumerate((st["Xlo"], st["Xhi"])):
                    sq = isb_p.tile([128, 4, NB], bf16, tag=f"sq{h}",
                                    name=f"sq{t}_{h}")
                    nc.scalar.activation(sq[:], X[:], SQUARE)
                    s = isb_p.tile([128, 2, NB], bf16, tag=f"s{h}", name=f"s{t}_{h}")
                    nc.gpsimd.tensor_tensor(s[:], sq[:, 0:2], sq[:, 2:4], ADD)
                    nc.gpsimd.tensor_tensor(I[:, h], s[:, 0], s[:, 1], ADD)
                st["I"] = I

            def eC(t):
                """corr matmuls -> phi psum bank; single bf16 copy (no dup)."""
                b = t // NQ
                st = state[t]
                I = st["I"]
                # phi bank [128, 4, NB]: q = ri*2 + half
                ph = ps_cor.tile([128, 4, 1, NB], f32, tag="ph", name=f"ph{t}")
                for ri in range(2):
                    for half in range(2):
                        q = ri * 2 + half
                        nc.tensor.matmul(ph[:, q], gslice(gall, b, ri, half, 0),
                                         I[:, 0], start=True, stop=False)
                        nc.tensor.matmul(ph[:, q], gslice(gall, b, ri, half, 1),
                                         I[:, 1], start=False, stop=True)
                phs = dup_p.tile([128, 4, 1, NB], bf16, tag="phs", name=f"phs{t}")
                nc.scalar.copy(phs[:], ph[:])
                st["phs"] = phs

            def eD(t):
                """U = j*P*phi .* X via broadcast reads of phi.

                X_h [128,4,NB] = (ri*m, j); T0 = nb*X, T1 = na*X (phi bcast
                over ri,m); Ur = T0[r]+T1[i], Ui = T0[i]-T1[r].
                """
                st = state[t]
                phs = st["phs"]
                U = {}
                for h, X in enumerate((st["Xlo"], st["Xhi"])):
                    naB = phs[:, 0 + h].broadcast_to([128, 4, NB])
                    nbB = phs[:, 2 + h].broadcast_to([128, 4, NB])
                    T0 = usb_p.tile([128, 4, NB], bf16, tag=f"T0{h}",
                                    name=f"T0{t}_{h}")
                    T1 = usb_p.tile([128, 4, NB], bf16, tag=f"T1{h}",
                                    name=f"T1{t}_{h}")
                    nc.vector.tensor_tensor(T0[:], X[:], nbB, MULT)
                    nc.vector.tensor_tensor(T1[:], X[:], naB, MULT)
                    Uh = usb_p.tile([128, 4, NB], bf16, tag=f"U{h}",
                                    name=f"U{t}_{h}")
                    nc.vector.tensor_tensor(Uh[:, 0:2], T0[:, 0:2], T1[:, 2:4], ADD)
                    nc.vector.tensor_tensor(Uh[:, 2:4], T0[:, 2:4], T1[:, 0:2], SUB)
                    U[h] = Uh
                st["U"] = U

            def eE(t):
                """dense IFFT: v_tc = sum_kc IDFT[tc,kc] @ U_kc, out copy + DMA."""
                st = state[t]
                U = st["U"]
                # single 2-bank psum [128, 2, 512]; used cols [0:464] per tc
                vp = ps_ift.tile([128, 2, 512], f32, tag="vp", name=f"vp{t}")
                for tcn in range(2):
                    mats = [(2 + tcn * 2 + kc) for kc in range(2)]
                    seq_r = []
                    seq_i = []
                    for kc in range(2):
                        mat = mats[kc]
                        Ur, Ui = U[kc][:, 0:2], U[kc][:, 2:4]
                        seq_r += [(wslice(wall, mat, 0), Ur),
                                  (wslice(wall, mat, 2), Ui)]
                        seq_i += [(wslice(wall, mat, 0), Ui),
                                  (wslice(wall, mat, 1), Ur)]
                    for ri, seq in enumerate((seq_r, seq_i)):
                        for i, (lhsT, rhs) in enumerate(seq):
                            nc.tensor.matmul(vp[:, tcn, ri * W2:(ri + 1) * W2],
                                             lhsT, rhs,
                                             start=(i == 0), stop=(i == 3))
                ob = osb_p.tile([128, 2, 2 * W2], bf16, tag="ob", name=f"ob{t}")
                nc.scalar.copy(ob[:], vp[:, :, :2 * W2])
                if t == NBLK - 1:
                    # last block: split across queues so the tail drains fast
                    nc.gpsimd.dma_start(vout_d[t, :, 0], ob[:, 0])
                    nc.sync.dma_start(vout_d[t, :, 1], ob[:, 1])
                else:
                    nc.gpsimd.dma_start(vout_d[t], ob[:])
                del state[t]

            # software pipeline: keep tensor fed two blocks ahead
            eDMA(0); eDMA(1); eDMA(2)
            eA(0); eB(0); eA(1); eB(1)
            for t in range(NBLK):
                eC(t)
                eD(t)
                if t + 3 < NBLK:
                    eDMA(t + 3)
                if t + 2 < NBLK:
                    eA(t + 2)
                    eB(t + 2)
                eE(t)

    nc.compile()
    return nc


def _run_device(xin_cores, wall_p, gall_p, trace=False):
    global _PROG, LAST_EXEC_NS, LAST_RESULTS
    from concourse.bass_utils import run_bass_kernel_spmd

    if _PROG is None:
        _PROG = _build_program()
    nc = _PROG
    in_maps = []
    for k in range(NCORES):
        in_maps.append({
            "xin": np.ascontiguousarray(xin_cores[k]).reshape(NBLK, 128, 8, NB),
            "wall": wall_p,
            "gall": gall_p,
        })
    kwargs = {}
    if trace:
        kwargs["trace"] = True
    res = run_bass_kernel_spmd(nc, in_maps, list(range(NCORES)), **kwargs)
    LAST_EXEC_NS = res.exec_time_ns
    LAST_RESULTS = res
    return [res.results[k]["vout"] for k in range(NCORES)]


def _emulate_device(xin_cores, wall, gall):
    """Numpy mirror of the device program (f32)."""
    outs = []
    for k in range(NCORES):
        xin = xin_cores[k].astype(np.float32)  # [NBLK, 128, 4, NM, NB]
        vout = np.empty((NBLK, 128, 2, 2, NM, NB), np.float32)
        for t in range(NBLK):
            b = t // NQ
            xer, xei = xin[t, :, 0].reshape(128, W2), xin[t, :, 1].reshape(128, W2)
            xor_, xoi = xin[t, :, 2].reshape(128, W2), xin[t, :, 3].reshape(128, W2)
            Er = wall[0].T @ xer + wall[2].T @ xei
            Ei = wall[0].T @ xei + wall[1].T @ xer
            Fr = wall[3].T @ xor_ + wall[5].T @ xoi
            Fi = wall[3].T @ xoi + wall[4].T @ xor_
            X = {0: (Er + Fr, Ei + Fi), 1: (Er - Fr, Ei - Fi)}
            I = {}
            for h in range(2):
                Xr, Xi = X[h]
                s = (Xr * Xr + Xi * Xi).reshape(128, NM, NB)
                I[h] = s[:, 0] + s[:, 1]
            gq = gall.reshape(B, 2, 2, 2, 128, 128)
            U = {}
            for h in range(2):
                na = gq[b, 0, h, 0].T @ I[0] + gq[b, 0, h, 1].T @ I[1]
                nb_ = gq[b, 1, h, 0].T @ I[0] + gq[b, 1, h, 1].T @ I[1]
                na2 = np.repeat(na[:, None, :], NM, 1).reshape(128, W2)
                nb2 = np.repeat(nb_[:, None, :], NM, 1).reshape(128, W2)
                Xr, Xi = X[h]
                U[h] = (nb2 * Xr + na2 * Xi, nb2 * Xi - na2 * Xr)
            for tcn in range(2):
                acc_r = np.zeros((128, W2), np.float32)
                acc_i = np.zeros((128, W2), np.float32)
                for kc in range(2):
                    mat = 2 + tcn * 2 + kc
                    Ur, Ui = U[kc]
                    acc_r += wall[3 * mat].T @ Ur + wall[3 * mat + 2].T @ Ui
                    acc_i += wall[3 * mat].T @ Ui + wall[3 * mat + 1].T @ Ur
                vout[t, :, tcn, 0] = acc_r.reshape(128, NM, NB)
                vout[t, :, tcn, 1] = acc_i.reshape(128, NM, NB)
        outs.append(vout)
    return outs


def kernel(x_real, x_imag, task_info, h_real, h_imag, _emulate=False, _trace=False):
    x_real = np.asarray(x_real, np.float32)
    x_imag = np.asarray(x_imag, np.float32)
    wall, gall, wall_p, gall_p = _build_const_matrices(h_real, h_imag, task_info)
    F = _frames_view(x_real, x_imag)
    xin_cores = _pack_inputs(F)
    if _emulate:
        vouts = _emulate_device(xin_cores, wall, gall)
    else:
        vouts = _run_device(xin_cores, wall_p, gall_p, trace=_trace)
    vfr = _unpack_outputs(vouts)
    yf = (F + vfr).transpose(0, 1, 2, 4, 3)   # [B, NM, 2, NFFT, FTOT]
    y = _overlap_add(yf)
    y = y[:, :, :, PAD:L - PAD]
    return np.ascontiguousarray(y.transpose(0, 3, 1, 2))
